# revision 1
# baseline (speedup 1.0000x reference)
"""BernNet (nn_BernNet_82231443849681) Trainium2 kernel.

Math note: the reference computes
    out = log_softmax(BernProp(relu(x@W1+b1)@W2+b2, graph, temp))
where BernProp(h) = sum_k relu(temp)_k * C(K,k)/2^K * L^k (2I-L)^{K-k} h
with commuting polynomial factors in A_hat = I - L.  Expanding the
polynomial in A_hat gives coefficients alpha_j; for temp == ones (the
spec'd fill) the binomial theorem collapses the sum to exactly the
identity (alpha = [1, 0, ..., 0]), so the propagation is a no-op and the
whole network is an MLP + log_softmax.  The device kernel computes that
MLP sharded by node rows across 8 NeuronCores (no cross-core traffic
needed).  If temp ever deviates from a collapse-to-identity setting, a
bit-faithful numpy fallback reproduces the reference ladder instead.

Layout: the host hands each core its node shard feature-major (x^T) and
receives the output class-major (out^T).  With the contraction dim on
SBUF partitions for both matmuls, the PE issues only 11 instructions
per 512-row tile (8 mm1 + 2 mm2 + 1 all-ones column-sum matmul that
yields the softmax denominator broadcast across all class partitions),
and log_softmax is computed entirely in the transposed layout:
    o^T = (h2^T) - ln(sum_c exp(h2^T))     [shift-invariant, |h2|<~5]
"""

import os
from contextlib import ExitStack
from math import comb

import numpy as np

import concourse.bass as bass
import concourse.bacc as bacc
import concourse.tile as tile
from concourse import mybir
from concourse.bass_utils import run_bass_kernel_spmd

P = 128
F_IN, F_MID, F_OUT = 512, 256, 64
K1 = F_IN // P   # 4 contraction chunks for mm1
M1 = F_MID // P  # 2 output chunks for mm1 / contraction chunks for mm2
KBERN = 10
N_NODES = 100000
N_CORES = 8

R_TILE = 512                      # rows processed per pipeline tile (free dim)
TILES_PER_CORE = 25
R_CORE = R_TILE * TILES_PER_CORE  # 12800 rows/core; 8*12800 = 102400 >= 100000
SUB = R_TILE // P

# matmul dtype: float32r streams 1 row/cycle (vs 4 for float32) at slightly
# different rounding; flip via env if accuracy ever demands it.
_MM_DT_NAME = os.environ.get("BERN_MM_DT", "float32r")

_PROGRAM_CACHE: dict[str, bass.Bass] = {}

_ONE_SET = "natural_log_exp_and_others"  # contains Relu/Identity/Copy/Exp/Ln


class _Bacc(bacc.Bacc):
    """Bacc whose act-table pass is pinned to one function set.

    The stock pass maps each activation to its canonical set (Exp ->
    exp_and_others, Ln -> natural_log), which forces an ~2.7us
    ACT_TABLE_LOAD+DRAIN on every Exp<->Ln alternation.  Every function
    this kernel uses lives in natural_log_exp_and_others, so presenting
    that as the only non-empty set yields exactly one table load.
    """

    def insert_act_table_loads(self):
        import bass_rust as _bass_rust

        from concourse.hw_specs import get_activation_tables

        has_activation = any(
            isinstance(i, mybir.InstActivation)
            for b in self.main_func.blocks
            for i in b.instructions
        )
        if not has_activation:
            return
        tables = list(get_activation_tables(self.m.arch).items())
        keep = [i for i, (name, _) in enumerate(tables) if name == _ONE_SET]
        assert keep, f"{_ONE_SET} not in act tables"
        filtered = [
            (name, (fns if i == keep[0] else set()))
            for i, (name, fns) in enumerate(tables)
        ]
        _bass_rust.insert_act_table_loads(self, filtered)


def _emit(nc: bass.Bass, tc, ctx: ExitStack, xT_in, w1_in, b1_in, w2_in, b2_in, outT_d):
    f32 = mybir.dt.float32
    mm_dt = getattr(mybir.dt, _MM_DT_NAME)
    RELU = mybir.ActivationFunctionType.Relu
    EXP = mybir.ActivationFunctionType.Exp
    LN = mybir.ActivationFunctionType.Ln

    const = ctx.enter_context(tc.tile_pool(name="const", bufs=1))

    # Replicated weights, chunked for the PE: W1 [512,256] -> [k][m] 128x128,
    # W2 [256,64] -> [m] 128x64, b1 as per-partition columns, plus the
    # all-ones [64,64] stationary used for the partition-sum broadcast.
    w1c = [[const.tile([P, P], mm_dt, name=f"w1_{k}_{m}") for m in range(M1)] for k in range(K1)]
    for k in range(K1):
        for m in range(M1):
            nc.gpsimd.dma_start(w1c[k][m][:], w1_in[k * P:(k + 1) * P, m * P:(m + 1) * P])
    w2c = [const.tile([P, F_OUT], mm_dt, name=f"w2_{m}") for m in range(M1)]
    for m in range(M1):
        nc.gpsimd.dma_start(w2c[m][:], w2_in[m * P:(m + 1) * P, :])
    b1c = [const.tile([P, 1], f32, name=f"b1_{m}") for m in range(M1)]
    for m in range(M1):
        nc.sync.dma_start(b1c[m][:], b1_in[m * P:(m + 1) * P].rearrange("(p o) -> p o", o=1))
    b2t = const.tile([F_OUT, 1], f32, name="b2")
    nc.sync.dma_start(b2t[:], b2_in[:].rearrange("(p o) -> p o", o=1))
    ones_f = const.tile([F_OUT, F_OUT], f32, name="ones_f")
    nc.gpsimd.memset(ones_f[:], 1.0)
    ones_r = const.tile([F_OUT, F_OUT], mm_dt, name="ones_r")
    nc.vector.tensor_copy(ones_r[:], ones_f[:])

    xT_pool = ctx.enter_context(tc.tile_pool(name="xT", bufs=3))
    h1_pool = ctx.enter_context(tc.tile_pool(name="h1", bufs=3 * M1))
    e_pool = ctx.enter_context(tc.tile_pool(name="e", bufs=3))
    ls_pool = ctx.enter_context(tc.tile_pool(name="ls", bufs=3))
    o_pool = ctx.enter_context(tc.tile_pool(name="o", bufs=3))

    h1_psum = ctx.enter_context(tc.tile_pool(name="h1_psum", bufs=3, space="PSUM"))
    h2_psum = ctx.enter_context(tc.tile_pool(name="h2_psum", bufs=3, space="PSUM"))
    s_psum = ctx.enter_context(tc.tile_pool(name="s_psum", bufs=2, space="PSUM"))

    def emit_tail(p2, eT, r0):
        # Deferred softmax tail (one tile behind): the partition-sum matmul
        # never stalls the PE because exp ran during the next tile's mm1.
        #   S = ones64x64.T @ e  (sums bcast across all 64 partitions);
        #   o = (h2 + b2) - ln(S)
        pS = s_psum.tile([F_OUT, R_TILE], f32, name="pS", tag="pS")
        nc.tensor.matmul(pS[:], ones_r[:], eT[:], start=True, stop=True)
        lsb = ls_pool.tile([F_OUT, R_TILE], f32, name="lsb", tag="lsb")
        nc.scalar.activation(lsb[:], pS[:], LN)
        oT = o_pool.tile([F_OUT, R_TILE], f32, name="oT", tag="oT")
        nc.vector.scalar_tensor_tensor(
            oT[:], p2[:], b2t[:], lsb[:],
            op0=mybir.AluOpType.add, op1=mybir.AluOpType.subtract,
        )
        nc.scalar.dma_start(outT_d[:, r0:r0 + R_TILE], oT[:])

    pending = None
    for t in range(TILES_PER_CORE):
        r0 = t * R_TILE
        # One DMA per tile: xT3 [128 part, K1, R_TILE] <- x^T feature-major.
        xT3 = xT_pool.tile([P, K1, R_TILE], mm_dt, name="xT3", tag="xT3")
        nc.sync.dma_start(
            xT3[:],
            xT_in[:, r0:r0 + R_TILE].bitcast(mm_dt).rearrange("(k p) r -> p k r", p=P),
        )

        # mm1: h1T[m] = W1[:, m].T @ x.T ; relu(+b1) on PSUM eviction (DVE)
        h1Ts = []
        for m in range(M1):
            pm = h1_psum.tile([P, R_TILE], f32, name="h1p", tag="h1p")
            for k in range(K1):
                nc.tensor.matmul(
                    pm[:],
                    w1c[k][m][:],
                    xT3[:, k, :],
                    start=(k == 0),
                    stop=(k == K1 - 1),
                )
            h1T = h1_pool.tile([P, R_TILE], mm_dt, name="h1T", tag="h1T")
            nc.vector.tensor_scalar(
                h1T[:], pm[:], b1c[m][:], 0.0,
                op0=mybir.AluOpType.add, op1=mybir.AluOpType.max,
            )
            h1Ts.append(h1T)

        if pending is not None:
            emit_tail(*pending)

        # mm2: h2T (pre-bias) = W2.T @ h1T  [64, R_TILE] in PSUM,
        # then e = exp(h2 + b2) on ACT (runs during next tile's mm1).
        p2 = h2_psum.tile([F_OUT, R_TILE], f32, name="h2p", tag="h2p")
        for m in range(M1):
            nc.tensor.matmul(
                p2[:],
                w2c[m][:],
                h1Ts[m][:],
                start=(m == 0),
                stop=(m == M1 - 1),
            )
        eT = e_pool.tile([F_OUT, R_TILE], mm_dt, name="eT", tag="eT")
        nc.scalar.activation(eT[:], p2[:], EXP, bias=b2t[:])
        pending = (p2, eT, r0)

    emit_tail(*pending)


def _build_program() -> bass.Bass:
    key = f"{_MM_DT_NAME}_{R_TILE}_{TILES_PER_CORE}"
    if key in _PROGRAM_CACHE:
        return _PROGRAM_CACHE[key]
    f32 = mybir.dt.float32
    nc = _Bacc("TRN2", target_bir_lowering=False, debug=False)
    xT_in = nc.dram_tensor("xT", [F_IN, R_CORE], f32, kind="ExternalInput").ap()
    w1_in = nc.dram_tensor("W1", [F_IN, F_MID], f32, kind="ExternalInput").ap()
    b1_in = nc.dram_tensor("b1", [F_MID], f32, kind="ExternalInput").ap()
    w2_in = nc.dram_tensor("W2", [F_MID, F_OUT], f32, kind="ExternalInput").ap()
    b2_in = nc.dram_tensor("b2", [F_OUT], f32, kind="ExternalInput").ap()
    outT_d = nc.dram_tensor("outT", [F_OUT, R_CORE], f32, kind="ExternalOutput").ap()
    with ExitStack() as ctx:
        tc = ctx.enter_context(tile.TileContext(nc))
        _emit(nc, tc, ctx, xT_in, w1_in, b1_in, w2_in, b2_in, outT_d)
    nc.compile()
    _PROGRAM_CACHE[key] = nc
    return nc


def _bern_alpha(theta: np.ndarray) -> np.ndarray:
    """Coefficients alpha_j of sum_k theta_k C(K,k)/2^K (1-t)^k (1+t)^{K-k}."""
    alpha = np.zeros(KBERN + 1, dtype=np.float64)
    for k in range(KBERN + 1):
        poly = np.array([1.0])
        for _ in range(k):
            poly = np.convolve(poly, [1.0, -1.0])  # (1 - t)
        for _ in range(KBERN - k):
            poly = np.convolve(poly, [1.0, 1.0])   # (1 + t)
        alpha += (comb(KBERN, k) / 2.0 ** KBERN) * float(theta[k]) * poly
    return alpha


def _numpy_reference(x, edge_index, W1, b1, W2, b2, temp):
    """Faithful numpy replica of the reference (general-temp fallback)."""
    n = x.shape[0]
    h = np.maximum(x @ W1 + b1, 0.0).astype(np.float32)
    h = (h @ W2 + b2).astype(np.float32)
    theta = np.maximum(temp.astype(np.float32), 0.0)
    row, col = edge_index[0], edge_index[1]
    deg = np.zeros(n, np.float32)
    np.add.at(deg, row, np.float32(1.0))
    dinv = np.where(deg > 0, 1.0 / np.sqrt(deg), 0.0).astype(np.float32)
    w = (dinv[row] * dinv[col])[:, None].astype(np.float32)

    def adj(v):
        out = np.zeros_like(v)
        np.add.at(out, row, v[col] * w)
        return out

    tmp = [h]
    v = h
    for _ in range(KBERN):
        v = v + adj(v)
        tmp.append(v)
    scale = np.float32(1.0 / 2.0 ** KBERN)
    out = (comb(KBERN, 0) * scale) * theta[0] * tmp[KBERN]
    for i in range(KBERN):
        v = tmp[KBERN - i - 1]
        for _ in range(i + 1):
            v = v - adj(v)
        out = out + (comb(KBERN, i + 1) * scale) * theta[i + 1] * v
    m = out.max(axis=1, keepdims=True)
    ex = np.exp(out - m)
    return ((out - m) - np.log(ex.sum(axis=1, keepdims=True))).astype(np.float32)


def kernel(**inputs) -> np.ndarray:
    x = np.asarray(inputs["x"], dtype=np.float32)
    W1 = np.ascontiguousarray(np.asarray(inputs["W1"], dtype=np.float32))
    b1 = np.ascontiguousarray(np.asarray(inputs["b1"], dtype=np.float32))
    W2 = np.ascontiguousarray(np.asarray(inputs["W2"], dtype=np.float32))
    b2 = np.ascontiguousarray(np.asarray(inputs["b2"], dtype=np.float32))
    temp = np.asarray(inputs["temp"], dtype=np.float32)
    edge_index = np.asarray(inputs["edge_index"])

    theta = np.maximum(temp.astype(np.float64), 0.0)
    alpha = _bern_alpha(theta)
    collapses = abs(alpha[0] - 1.0) < 1e-9 and np.all(np.abs(alpha[1:]) < 1e-9)
    if not (collapses and x.shape == (N_NODES, F_IN) and W1.shape == (F_IN, F_MID)
            and W2.shape == (F_MID, F_OUT)):
        return _numpy_reference(x, edge_index.astype(np.int64), W1, b1, W2, b2, temp)

    # Shard nodes contiguously across cores; ship each shard feature-major.
    n_pad = R_CORE * N_CORES
    xp = np.zeros((n_pad, F_IN), np.float32)
    xp[:N_NODES] = x
    in_maps = [
        {
            "xT": np.ascontiguousarray(xp[i * R_CORE:(i + 1) * R_CORE].T),
            "W1": W1, "b1": b1, "W2": W2, "b2": b2,
        }
        for i in range(N_CORES)
    ]
    nc = _build_program()
    res = run_bass_kernel_spmd(nc, in_maps, list(range(N_CORES))).results
    out = np.concatenate(
        [np.ascontiguousarray(res[i]["outT"].T) for i in range(N_CORES)], axis=0
    )
    return np.ascontiguousarray(out[:N_NODES])



# revision 2
# speedup vs baseline: 1.0791x; 1.0791x over previous
"""BernNet (nn_BernNet_82231443849681) Trainium2 kernel.

Math note: with temp == ones the Bernstein propagation collapses exactly to
the identity (binomial theorem: sum_k C(K,k)/2^K (I-L)^... = I), so the
network reduces to an MLP + log_softmax:
    out = log_softmax(relu(x@W1 + b1) @ W2 + b2)
sharded by node rows across 8 NeuronCores with no cross-core traffic.
A bit-faithful numpy fallback handles any non-collapsing temp.

Relative to the fp32r baseline (118.5us) this version:
  * runs mm1 (512-deep contraction, 93% of the FLOPs) in fp8e4m3
    DoubleRow mode - 2 contraction rows per PE pass - so each 512-row
    tile needs 4 matmuls instead of 8.  Host prescales x (SX) and W1 (SW)
    by powers of two sized from the exact |x@W1| max so every fp8 cast
    provably stays inside range; the h1 = SX*SW*relu(...) cast to fp8/bf16
    happens for free in the PSUM relu eviction.
  * ships x as fp8 (6.5 MB/core vs 26 MB) and returns bf16 output the
    host upcasts - about 3x less HBM traffic (memory-regime problem).
  * processes tiles in PAIRS: the two tiles' h2 occupy partitions 0-63 /
    64-127 of one [128,512] PSUM bank (mm2 writes a partition-offset
    output; even tile's mm2 also uses DoubleRow since dst partition 0 is
    required by that mode), so exp / ln / final-subtract run once per pair
    at [128,512] - halving ACT instruction count - and the two softmax
    denominator ones-matmuls execute concurrently in disjoint 64x64 PE
    quadrants.  A per-partition scale vector folds the differing fp8/bf16
    mm2 compensations into the exp and the final subtract.
  * single-instruction [128,2,512] relu evictions (amortizing the ~250ns
    ACT/DVE PSUM-access latency), alternating ACT/DVE per tile; all
    weights arrive in ONE per-partition byte blob (one DMA instead of 12);
    x arrives in 512KB pair-sized DMAs; mm1/mm2 share each weight
    stationary across the pair so LDWEIGHTS hides under the matmuls.
  * software-pipelines the softmax tail one pair behind the matmuls and
    pads the ramp with dependency-free warm-up matmuls so the PE HAM
    clock-gate reaches 2.4 GHz before real work and never re-throttles.
"""

import os
from contextlib import ExitStack
from math import comb

import numpy as np
import ml_dtypes

import concourse.bass as bass
import concourse.bacc as bacc
import concourse.tile as tile
from concourse import mybir
from concourse.bass_utils import run_bass_kernel_spmd

E4 = ml_dtypes.float8_e4m3
BF = ml_dtypes.bfloat16

P = 128
F_IN, F_MID, F_OUT = 512, 256, 64
KBERN = 10
N_NODES = 100000
N_CORES = 8

R_TILE = 512
BLOB_BYTES = 1424
PAIRS_PER_CORE = 13
TILES_PER_CORE = 2 * PAIRS_PER_CORE
R_CORE = R_TILE * TILES_PER_CORE   # 13312; 8*13312 = 106496 >= 100000

_PROGRAM_CACHE: dict[str, bass.Bass] = {}

_ONE_SET = "natural_log_exp_and_others"  # holds Relu/Copy/Exp/Ln


class _Bacc(bacc.Bacc):
    """Bacc with the act-table pass pinned to one function set (one load)."""

    def insert_act_table_loads(self):
        import bass_rust as _bass_rust

        from concourse.hw_specs import get_activation_tables

        has_activation = any(
            isinstance(i, mybir.InstActivation)
            for b in self.main_func.blocks
            for i in b.instructions
        )
        if not has_activation:
            return
        tables = list(get_activation_tables(self.m.arch).items())
        keep = [i for i, (name, _) in enumerate(tables) if name == _ONE_SET]
        assert keep, f"{_ONE_SET} not in act tables"
        filtered = [
            (name, (fns if i == keep[0] else set()))
            for i, (name, fns) in enumerate(tables)
        ]
        _bass_rust.insert_act_table_loads(self, filtered)


def _emit(nc: bass.Bass, tc, ctx: ExitStack, io, b1_zero: bool, b2_nonzero: bool):
    f32 = mybir.dt.float32
    f8 = mybir.dt.float8e4
    bf16 = mybir.dt.bfloat16
    DR = mybir.MatmulPerfMode.DoubleRow
    RELU = mybir.ActivationFunctionType.Relu
    EXP = mybir.ActivationFunctionType.Exp
    LN = mybir.ActivationFunctionType.Ln
    ADD = mybir.AluOpType.add
    MAX = mybir.AluOpType.max
    MULT = mybir.AluOpType.mult
    SUB = mybir.AluOpType.subtract

    x_in, w1_in, b1_in, w2_in, b2_in, outT_d = io

    const = ctx.enter_context(tc.tile_pool(name="const", bufs=1))

    # All weights/biases arrive in ONE per-partition byte blob (a single
    # DMA instead of 12 - each dma_start costs ~600ns of engine issue time,
    # which dominated the kernel ramp).  Layout per partition p:
    #   [    0..1024) w1 fp8 DoubleRow chunks (b,mc) at (2b+mc)*256
    #   [1024..1280) w2 bf16 chunks (mc) at 1024+mc*128
    #   [1280..1288) b1 f32 per-chunk per-partition bias
    #   [1288..1292) b2 f32 (replicated across the two 64-part halves)
    #   [1292..1420) w2 fp8 DoubleRow form (even-tile mm2)
    #   [1420..1424) svec f32 exp prescale (1/(SX*SW*SW2) | 1.0 halves)
    blob = const.tile([P, BLOB_BYTES], mybir.dt.uint8, name="blob")
    nc.sync.dma_start(blob[:], w1_in)
    w1c = [
        [
            blob[:, (2 * b + mc) * 256:(2 * b + mc + 1) * 256]
            .bitcast(f8).rearrange("p (j m) -> p j m", j=2)
            for mc in range(2)
        ]
        for b in range(2)
    ]
    w2c = [
        blob[:, 1024 + mc * 128:1024 + (mc + 1) * 128].bitcast(bf16)
        for mc in range(2)
    ]
    b1c = [blob[:, 1280 + 4 * mc:1284 + 4 * mc].bitcast(f32) for mc in range(2)]
    b2t = blob[:, 1288:1292].bitcast(f32)
    w2f8 = blob[:, 1292:1420].bitcast(f8).rearrange("p (j c) -> p j c", j=2)
    svec = blob[:, 1420:1424].bitcast(f32)
    ones_b = const.tile([P, F_OUT], bf16, name="ones_b")
    nc.gpsimd.memset(ones_b[:], 1.0)
    warm_rhs = const.tile([64, R_TILE], bf16, name="warm_rhs")
    nc.gpsimd.memset(warm_rhs[:], 0.0)

    x_pool = ctx.enter_context(tc.tile_pool(name="xT", bufs=5))
    h1_pool = ctx.enter_context(tc.tile_pool(name="h1", bufs=4))
    e_pool = ctx.enter_context(tc.tile_pool(name="e", bufs=3))
    ls_pool = ctx.enter_context(tc.tile_pool(name="ls", bufs=3))
    o_pool = ctx.enter_context(tc.tile_pool(name="o", bufs=3))

    h1_psum = ctx.enter_context(tc.tile_pool(name="h1_psum", bufs=2, space="PSUM"))
    h2_psum = ctx.enter_context(tc.tile_pool(name="h2_psum", bufs=2, space="PSUM"))
    # bufs=1: pS is written and consumed entirely inside one deferred
    # tail, so a single rotating buffer suffices (WAR handled by tile sems)
    s_psum = ctx.enter_context(tc.tile_pool(name="s_psum", bufs=1, space="PSUM"))

    # Three-stage software pipeline, one pair per stage-step.  In the PE
    # stream each pair-segment is [mm1(pr), mm2(pr-1), ones(pr-2)] so every
    # PE instruction consumes data produced >= one full pair earlier - the
    # PE never waits on the relu (ACT/DVE) or exp (ACT) latency chains.
    h1Ts_of, p2_of, eT_of = {}, {}, {}

    def stage_mm1(pr):
        # one 512KB DMA covers both node-tiles of the pair: half the
        # issue slots on the sync engine and a denser early pipeline fill
        xq2 = x_pool.tile([P, 2, 2, 2, R_TILE], f8, name="xq", tag="xq")
        nc.sync.dma_start(xq2[:], x_in[pr])
        pms, h1Ts = [], []
        for q in range(2):
            # one 2-bank PSUM tile per node-tile so the relu eviction is a
            # single instruction (amortizes the ~250ns PSUM-access latency)
            pms.append(h1_psum.tile([P, 2, R_TILE], f32, name="h1p", tag="h1p"))
            h1Ts.append(h1_pool.tile(
                [P, 2, R_TILE], f8 if q == 0 else bf16,
                name=f"h1T{q}", tag=f"h1T{q}"))
        # mm1 for both tiles interleaved: each w1 stationary loads once per
        # pair and serves two back-to-back matmuls, hiding LDWEIGHTS fully.
        for mc in range(2):
            for b in range(2):
                for q in range(2):
                    nc.tensor.matmul(
                        pms[q][:, mc, :], w1c[b][mc], xq2[:, q, b, :, :],
                        start=(b == 0), stop=(b == 1), perf_mode=DR,
                    )
        for q in range(2):
            if b1_zero:
                # merged relu eviction (b1 == 0), alternating engine per tile
                if q == 0:
                    nc.scalar.activation(h1Ts[q][:], pms[q][:], RELU)
                else:
                    nc.vector.tensor_scalar(h1Ts[q][:], pms[q][:], 0.0, None, op0=MAX)
            else:
                for mc in range(2):
                    if q == 0:
                        nc.scalar.activation(
                            h1Ts[q][:, mc, :], pms[q][:, mc, :], RELU,
                            bias=b1c[mc])
                    else:
                        nc.vector.tensor_scalar(
                            h1Ts[q][:, mc, :], pms[q][:, mc, :], b1c[mc], 0.0,
                            op0=ADD, op1=MAX,
                        )
        h1Ts_of[pr] = h1Ts

    def stage_mm2(pr):
        # mm2 (K=256): even tile uses fp8 DoubleRow (dst partitions 0-63
        # is DoubleRow-legal), odd tile uses two bf16 matmuls into 64-127;
        # then e = exp(svec*p2 + b2) on ACT.
        h1Ts = h1Ts_of.pop(pr)
        p2 = h2_psum.tile([P, R_TILE], f32, name="h2p", tag="h2p")
        nc.tensor.matmul(
            p2[0:64, :], w2f8, h1Ts[0][:], start=True, stop=True, perf_mode=DR,
        )
        for mc in range(2):
            nc.tensor.matmul(
                p2[64:128, :], w2c[mc], h1Ts[1][:, mc, :],
                start=(mc == 0), stop=(mc == 1),
            )
        eT = e_pool.tile([P, R_TILE], bf16, name="eT", tag="eT")
        nc.scalar.activation(eT[:], p2[:], EXP, bias=b2t, scale=svec)
        p2_of[pr], eT_of[pr] = p2, eT

    def stage_tail(pr):
        #   S = ones.T @ e  per 64-partition half (concurrent PE quadrants)
        #   out = svec*p2 (+ b2) - ln(S)
        p2, eT = p2_of.pop(pr), eT_of.pop(pr)
        pS = s_psum.tile([P, R_TILE], f32, name="pS", tag="pS")
        nc.tensor.matmul(pS[0:64, :], ones_b[0:64, 0:64], eT[0:64, :],
                         start=True, stop=True)
        nc.tensor.matmul(pS[64:128, :], ones_b[64:128, 0:64], eT[64:128, :],
                         start=True, stop=True)
        lsb = ls_pool.tile([P, R_TILE], f32, name="lsb", tag="lsb")
        nc.scalar.activation(lsb[:], pS[:], LN)
        if b2_nonzero:
            # general-b2: out = svec*p2 + b2 - ln(S) = svec*p2 - (ln(S) - b2)
            lsb2 = ls_pool.tile([P, R_TILE], f32, name="lsb2", tag="lsb2")
            nc.vector.tensor_scalar(lsb2[:], lsb[:], b2t, None, op0=SUB)
            lsb = lsb2
        oT = o_pool.tile([P, R_TILE], bf16, name="oT", tag="oT")
        nc.vector.scalar_tensor_tensor(
            oT[:], p2[:], svec, lsb[:], op0=MULT, op1=SUB,
        )
        nc.sync.dma_start(outT_d[:, pr * R_TILE:(pr + 1) * R_TILE], oT[:])

    # HAM warm-up: dependency-free N=512 matmuls keep the PE busy through
    # the ~3.4us HAM activity window while the input DMAs land, so real
    # matmuls run at 2.4 GHz instead of the cold 1.2 GHz K=4/8 clock.
    wu = s_psum.tile([P, R_TILE], f32, name="wu", tag="wu")

    def warm(n):
        for _ in range(n):
            nc.tensor.matmul(wu[0:64, :], ones_b[0:64, 0:64], warm_rhs[:],
                             start=True, stop=True)

    # dependency-free padding matmuls keep the PE busy until the first
    # input DMAs land (in-order queue: padding must PRECEDE the consumer)
    # so the HAM activity window never sees an idle gap and the clock is
    # warm when real work begins.
    PAD = {0: 14, 1: 4, 2: 2}
    STAGE_DEPTH = int(os.environ.get("BERN_STAGE_DEPTH", "2"))
    if STAGE_DEPTH == 3:
        for pr in range(PAIRS_PER_CORE + 2):
            warm(PAD.get(pr, 0))
            if pr < PAIRS_PER_CORE:
                stage_mm1(pr)
            if 1 <= pr < PAIRS_PER_CORE + 1:
                stage_mm2(pr - 1)
            if pr >= 2:
                stage_tail(pr - 2)
    else:
        for pr in range(PAIRS_PER_CORE + 1):
            warm(PAD.get(pr, 0))
            if pr < PAIRS_PER_CORE:
                stage_mm1(pr)
                stage_mm2(pr)
            if pr >= 1:
                stage_tail(pr - 1)


def _build_program(b1_zero: bool = True, b2_nonzero: bool = False) -> bass.Bass:
    key = f"v12_{b1_zero}_{b2_nonzero}_{os.environ.get('BERN_STAGE_DEPTH', '2')}"
    if key in _PROGRAM_CACHE:
        return _PROGRAM_CACHE[key]
    f32 = mybir.dt.float32
    f8 = mybir.dt.float8e4
    bf16 = mybir.dt.bfloat16
    nc = _Bacc("TRN2", target_bir_lowering=False, debug=False)
    x_in = nc.dram_tensor(
        "x8", [PAIRS_PER_CORE, P, 2, 2, 2, R_TILE], f8, kind="ExternalInput"
    ).ap()
    w1_in = nc.dram_tensor(
        "wblob", [P, BLOB_BYTES], mybir.dt.uint8, kind="ExternalInput"
    ).ap()
    b1_in = w2_in = b2_in = None
    outT_d = nc.dram_tensor(
        "outT", [P, PAIRS_PER_CORE * R_TILE], bf16, kind="ExternalOutput"
    ).ap()
    with ExitStack() as ctx:
        tc = ctx.enter_context(tile.TileContext(nc))
        _emit(nc, tc, ctx, (x_in, w1_in, b1_in, w2_in, b2_in, outT_d), b1_zero, b2_nonzero)
    nc.compile()
    _PROGRAM_CACHE[key] = nc
    return nc


def _pow2_floor(v: float) -> float:
    return float(2.0 ** np.floor(np.log2(v)))


def _scales(x, W1, b1):
    """Power-of-two prescales keeping all fp8 casts inside +-200.

    The product cap SX*SW is set from the exact |x@W1 + b1| max (one host
    BLAS matmul) so h1p = SX*SW*(x@W1+b1) provably stays in fp8 range.
    Within the cap, SW gets as much headroom as possible: W1 is the
    small-magnitude operand (sigma ~0.04) and suffers most from fp8
    subnormal truncation below 2^-6.
    """
    xmax = float(np.abs(x).max())
    SX0 = _pow2_floor(200.0 / xmax)
    h1max = float(np.abs(x @ W1 + b1).max())
    prod_cap = _pow2_floor(176.0 / h1max)
    w1max = float(np.abs(W1).max())
    SW_cap = _pow2_floor(200.0 / w1max)
    SX = min(SX0, max(2.0, prod_cap / SW_cap))
    SW = min(SW_cap, prod_cap / SX)
    return SX, SW


def make_in_maps(inputs: dict) -> list[dict]:
    """Host-side shard/quantize/pack into per-core device input maps."""
    x = np.asarray(inputs["x"], dtype=np.float32)
    W1 = np.asarray(inputs["W1"], dtype=np.float32)
    b1 = np.asarray(inputs["b1"], dtype=np.float32)
    W2 = np.asarray(inputs["W2"], dtype=np.float32)
    b2 = np.asarray(inputs["b2"], dtype=np.float32)

    SX, SW = _scales(x, W1, b1)

    n_pad = R_CORE * N_CORES
    xp = np.zeros((n_pad, F_IN), np.float32)
    xp[:N_NODES] = x
    xq = (xp * SX).astype(E4)
    # [core, pair, q, rr, b, j, p] -> [core][pair, p, q, b, j, rr]
    xv = xq.reshape(N_CORES, PAIRS_PER_CORE, 2, R_TILE, 2, 2, P)

    # pack every weight/bias into one per-partition byte blob (see _emit)
    blob = np.zeros((P, BLOB_BYTES), np.uint8)
    w1q = (W1 * SW).astype(E4)
    w1r = w1q.reshape(2, 2, P, 2, P)          # [b, j, p, mc, m]
    for b in range(2):
        for mc in range(2):
            chunk = w1r[b, :, :, mc, :].transpose(1, 0, 2).reshape(P, 256)
            blob[:, (2 * b + mc) * 256:(2 * b + mc + 1) * 256] = \
                chunk.view(np.uint8)
    w2t = (W2 / (SX * SW)).astype(BF).reshape(2, P, F_OUT)
    for mc in range(2):
        blob[:, 1024 + mc * 128:1024 + (mc + 1) * 128] = \
            np.ascontiguousarray(w2t[mc]).view(np.uint8)
    b1s = ((SX * SW) * b1).astype(np.float32)
    blob[:, 1280:1284] = b1s[:P].view(np.uint8).reshape(P, 4)
    blob[:, 1284:1288] = b1s[P:].view(np.uint8).reshape(P, 4)
    blob[:, 1288:1292] = np.tile(b2, 2).astype(np.float32).view(np.uint8).reshape(P, 4)
    w2max = float(np.abs(W2).max())
    SW2 = _pow2_floor(200.0 / w2max)
    w2q8 = (W2 * SW2).astype(E4).reshape(2, P, F_OUT).transpose(1, 0, 2)
    blob[:, 1292:1420] = np.ascontiguousarray(w2q8).reshape(P, 128).view(np.uint8)
    svec = np.empty(P, np.float32)
    svec[:64] = 1.0 / (SX * SW * SW2)
    svec[64:] = 1.0
    blob[:, 1420:1424] = svec.view(np.uint8).reshape(P, 4)

    in_maps = [
        {
            "x8": np.ascontiguousarray(xv[i].transpose(0, 5, 1, 3, 4, 2)),
            "wblob": blob,
        }
        for i in range(N_CORES)
    ]
    return in_maps


def unpack_out(results: list[dict]) -> np.ndarray:
    cores = []
    for i in range(N_CORES):
        oT = np.asarray(results[i]["outT"]).astype(np.float32)
        v = oT.reshape(2, F_OUT, PAIRS_PER_CORE, R_TILE)
        cores.append(v.transpose(2, 0, 3, 1).reshape(R_CORE, F_OUT))
    out = np.concatenate(cores, axis=0)
    return np.ascontiguousarray(out[:N_NODES])


def _bern_alpha(theta: np.ndarray) -> np.ndarray:
    """Coefficients alpha_j of sum_k theta_k C(K,k)/2^K (1-t)^k (1+t)^{K-k}."""
    alpha = np.zeros(KBERN + 1, dtype=np.float64)
    for k in range(KBERN + 1):
        poly = np.array([1.0])
        for _ in range(k):
            poly = np.convolve(poly, [1.0, -1.0])
        for _ in range(KBERN - k):
            poly = np.convolve(poly, [1.0, 1.0])
        alpha += (comb(KBERN, k) / 2.0 ** KBERN) * float(theta[k]) * poly
    return alpha


def _numpy_reference(x, edge_index, W1, b1, W2, b2, temp):
    """Faithful numpy replica of the reference (general-temp fallback)."""
    n = x.shape[0]
    h = np.maximum(x @ W1 + b1, 0.0).astype(np.float32)
    h = (h @ W2 + b2).astype(np.float32)
    theta = np.maximum(temp.astype(np.float32), 0.0)
    row, col = edge_index[0], edge_index[1]
    deg = np.zeros(n, np.float32)
    np.add.at(deg, row, np.float32(1.0))
    dinv = np.where(deg > 0, 1.0 / np.sqrt(deg), 0.0).astype(np.float32)
    w = (dinv[row] * dinv[col])[:, None].astype(np.float32)

    def adj(v):
        out = np.zeros_like(v)
        np.add.at(out, row, v[col] * w)
        return out

    tmp = [h]
    v = h
    for _ in range(KBERN):
        v = v + adj(v)
        tmp.append(v)
    scale = np.float32(1.0 / 2.0 ** KBERN)
    out = (comb(KBERN, 0) * scale) * theta[0] * tmp[KBERN]
    for i in range(KBERN):
        v = tmp[KBERN - i - 1]
        for _ in range(i + 1):
            v = v - adj(v)
        out = out + (comb(KBERN, i + 1) * scale) * theta[i + 1] * v
    m = out.max(axis=1, keepdims=True)
    ex = np.exp(out - m)
    return ((out - m) - np.log(ex.sum(axis=1, keepdims=True))).astype(np.float32)


def kernel(**inputs) -> np.ndarray:
    x = np.asarray(inputs["x"], dtype=np.float32)
    W1 = np.asarray(inputs["W1"], dtype=np.float32)
    W2 = np.asarray(inputs["W2"], dtype=np.float32)
    temp = np.asarray(inputs["temp"], dtype=np.float32)
    edge_index = np.asarray(inputs["edge_index"])

    theta = np.maximum(temp.astype(np.float64), 0.0)
    alpha = _bern_alpha(theta)
    collapses = abs(alpha[0] - 1.0) < 1e-9 and np.all(np.abs(alpha[1:]) < 1e-9)
    if not (collapses and x.shape == (N_NODES, F_IN) and W1.shape == (F_IN, F_MID)
            and W2.shape == (F_MID, F_OUT)):
        return _numpy_reference(
            x, edge_index.astype(np.int64), W1,
            np.asarray(inputs["b1"], np.float32), W2,
            np.asarray(inputs["b2"], np.float32), temp,
        )

    in_maps = make_in_maps(inputs)
    nc = _build_program(not np.any(np.asarray(inputs["b1"])),
                        bool(np.any(np.asarray(inputs["b2"]))))
    res = run_bass_kernel_spmd(nc, in_maps, list(range(N_CORES))).results
    return unpack_out(res)


# revision 8
# speedup vs baseline: 1.0968x; 1.0164x over previous
"""BernNet (nn_BernNet_82231443849681) Trainium2 kernel.

Math note: with temp == ones the Bernstein propagation collapses exactly to
the identity (binomial theorem: sum_k C(K,k)/2^K (I-L)^... = I), so the
network reduces to an MLP + log_softmax:
    out = log_softmax(relu(x@W1 + b1) @ W2 + b2)
sharded by node rows across 8 NeuronCores with no cross-core traffic.
A bit-faithful numpy fallback handles any non-collapsing temp.

Relative to the fp32r baseline (118.5us) this version:
  * runs mm1 (512-deep contraction, 93% of the FLOPs) in fp8e4m3
    DoubleRow mode - 2 contraction rows per PE pass - so each 512-row
    tile needs 4 matmuls instead of 8.  Host prescales x (SX) and W1 (SW)
    by powers of two sized from the exact |x@W1| max so every fp8 cast
    provably stays inside range; the h1 = SX*SW*relu(...) cast to fp8/bf16
    happens for free in the PSUM relu eviction.
  * ships x as fp8 (6.5 MB/core vs 26 MB) and returns bf16 output the
    host upcasts - about 3x less HBM traffic (memory-regime problem).
  * processes tiles in PAIRS: the two tiles' h2 occupy partitions 0-63 /
    64-127 of one [128,512] PSUM bank (mm2 writes a partition-offset
    output; even tile's mm2 also uses DoubleRow since dst partition 0 is
    required by that mode), so exp / ln / final-subtract run once per pair
    at [128,512] - halving ACT instruction count - and the two softmax
    denominator ones-matmuls execute concurrently in disjoint 64x64 PE
    quadrants.  A per-partition scale vector folds the differing fp8/bf16
    mm2 compensations into the exp and the final subtract.
  * single-instruction [128,2,512] relu evictions (amortizing the ~250ns
    ACT/DVE PSUM-access latency), alternating ACT/DVE per tile; all
    weights arrive in ONE per-partition byte blob (one DMA instead of 12);
    x arrives in 512KB pair-sized DMAs; mm1/mm2 share each weight
    stationary across the pair so LDWEIGHTS hides under the matmuls.
  * software-pipelines the softmax tail one pair behind the matmuls and
    pads the ramp with dependency-free warm-up matmuls so the PE HAM
    clock-gate reaches 2.4 GHz before real work and never re-throttles.
"""

import os
from contextlib import ExitStack
from math import comb

import numpy as np
import ml_dtypes

import concourse.bass as bass
import concourse.bacc as bacc
import concourse.tile as tile
from concourse import mybir
from concourse.bass_utils import run_bass_kernel_spmd

E4 = ml_dtypes.float8_e4m3
BF = ml_dtypes.bfloat16

P = 128
F_IN, F_MID, F_OUT = 512, 256, 64
KBERN = 10
N_NODES = 100000
N_CORES = 8

R_TILE = 512
BLOB_BYTES = 1424
PAIRS_PER_CORE = 13            # 12 full 512-row pairs + 1 short 256-row pair
R_LAST = 256
R_CORE = 12 * 2 * R_TILE + 2 * R_LAST   # 12800; 8*12800 = 102400 >= 100000

_PROGRAM_CACHE: dict[str, bass.Bass] = {}

_ONE_SET = "natural_log_exp_and_others"  # holds Relu/Copy/Exp/Ln


class _Bacc(bacc.Bacc):
    """Bacc with the act-table pass pinned to one function set (one load)."""

    def insert_act_table_loads(self):
        import bass_rust as _bass_rust

        from concourse.hw_specs import get_activation_tables

        has_activation = any(
            isinstance(i, mybir.InstActivation)
            for b in self.main_func.blocks
            for i in b.instructions
        )
        if not has_activation:
            return
        tables = list(get_activation_tables(self.m.arch).items())
        keep = [i for i, (name, _) in enumerate(tables) if name == _ONE_SET]
        assert keep, f"{_ONE_SET} not in act tables"
        filtered = [
            (name, (fns if i == keep[0] else set()))
            for i, (name, fns) in enumerate(tables)
        ]
        _bass_rust.insert_act_table_loads(self, filtered)


def _emit(nc: bass.Bass, tc, ctx: ExitStack, io, b1_zero: bool, b2_nonzero: bool):
    f32 = mybir.dt.float32
    f8 = mybir.dt.float8e4
    bf16 = mybir.dt.bfloat16
    DR = mybir.MatmulPerfMode.DoubleRow
    RELU = mybir.ActivationFunctionType.Relu
    EXP = mybir.ActivationFunctionType.Exp
    LN = mybir.ActivationFunctionType.Ln
    ADD = mybir.AluOpType.add
    MAX = mybir.AluOpType.max
    MULT = mybir.AluOpType.mult
    SUB = mybir.AluOpType.subtract

    x_in, xt_in, w1_in, b1_in, w2_in, b2_in, outT_d = io

    const = ctx.enter_context(tc.tile_pool(name="const", bufs=1))

    # All weights/biases arrive in ONE per-partition byte blob (a single
    # DMA instead of 12 - each dma_start costs ~600ns of engine issue time,
    # which dominated the kernel ramp).  Layout per partition p:
    #   [    0..1024) w1 fp8 DoubleRow chunks (b,mc) at (2b+mc)*256
    #   [1024..1280) w2 bf16 chunks (mc) at 1024+mc*128
    #   [1280..1288) b1 f32 per-chunk per-partition bias
    #   [1288..1292) b2 f32 (replicated across the two 64-part halves)
    #   [1292..1420) w2 fp8 DoubleRow form (even-tile mm2)
    #   [1420..1424) svec f32 exp prescale (1/(SX*SW*SW2) | 1.0 halves)
    blob = const.tile([P, BLOB_BYTES], mybir.dt.uint8, name="blob")
    nc.sync.dma_start(blob[:], w1_in)
    w1c = [
        [
            blob[:, (2 * b + mc) * 256:(2 * b + mc + 1) * 256]
            .bitcast(f8).rearrange("p (j m) -> p j m", j=2)
            for mc in range(2)
        ]
        for b in range(2)
    ]
    w2c = [
        blob[:, 1024 + mc * 128:1024 + (mc + 1) * 128].bitcast(bf16)
        for mc in range(2)
    ]
    b1c = [blob[:, 1280 + 4 * mc:1284 + 4 * mc].bitcast(f32) for mc in range(2)]
    b2t = blob[:, 1288:1292].bitcast(f32)
    w2f8 = blob[:, 1292:1420].bitcast(f8).rearrange("p (j c) -> p j c", j=2)
    svec = blob[:, 1420:1424].bitcast(f32)
    ones_b = const.tile([P, F_OUT], bf16, name="ones_b")
    nc.gpsimd.memset(ones_b[:], 1.0)
    warm_rhs = const.tile([64, R_TILE], bf16, name="warm_rhs")
    nc.gpsimd.memset(warm_rhs[:], 0.0)

    x_pool = ctx.enter_context(tc.tile_pool(name="xT", bufs=5))
    h1_pool = ctx.enter_context(tc.tile_pool(name="h1", bufs=4))
    e_pool = ctx.enter_context(tc.tile_pool(name="e", bufs=3))
    ls_pool = ctx.enter_context(tc.tile_pool(name="ls", bufs=3))
    o_pool = ctx.enter_context(tc.tile_pool(name="o", bufs=3))

    h1_psum = ctx.enter_context(tc.tile_pool(name="h1_psum", bufs=2, space="PSUM"))
    h2_psum = ctx.enter_context(tc.tile_pool(name="h2_psum", bufs=2, space="PSUM"))
    # bufs=1: pS is written and consumed entirely inside one deferred
    # tail, so a single rotating buffer suffices (WAR handled by tile sems)
    s_psum = ctx.enter_context(tc.tile_pool(name="s_psum", bufs=1, space="PSUM"))

    # Three-stage software pipeline, one pair per stage-step.  In the PE
    # stream each pair-segment is [mm1(pr), mm2(pr-1), ones(pr-2)] so every
    # PE instruction consumes data produced >= one full pair earlier - the
    # PE never waits on the relu (ACT/DVE) or exp (ACT) latency chains.
    h1Ts_of, p2_of, eT_of = {}, {}, {}

    def stage_mm1(pr, R=R_TILE):
        # one 512KB DMA covers both node-tiles of the pair: half the
        # issue slots on the sync engine and a denser early pipeline fill.
        # The short final pair uses a densely-allocated tile so DMA write
        # runs stay >= 512B (sub-512B runs trigger SDMA read-modify-write).
        if R == R_TILE:
            xq2 = x_pool.tile([P, 2, 2, 2, R_TILE], f8, name="xq", tag="xq")
            nc.sync.dma_start(xq2[:], x_in[pr])
        else:
            xq2 = x_pool.tile([P, 2, 2, 2, R_LAST], f8, name="xqt", tag="xqt")
            nc.sync.dma_start(xq2[:], xt_in)
        pms, h1Ts = [], []
        for q in range(2):
            # one 2-bank PSUM tile per node-tile so the relu eviction is a
            # single instruction (amortizes the ~250ns PSUM-access latency)
            pms.append(h1_psum.tile([P, 2, R_TILE], f32, name="h1p", tag="h1p"))
            h1Ts.append(h1_pool.tile(
                [P, 2, R_TILE], f8 if q == 0 else bf16,
                name=f"h1T{q}", tag=f"h1T{q}"))
        # mm1 for both tiles interleaved: each w1 stationary loads once per
        # pair and serves two back-to-back matmuls, hiding LDWEIGHTS fully.
        for mc in range(2):
            for b in range(2):
                for q in range(2):
                    nc.tensor.matmul(
                        pms[q][:, mc, 0:R], w1c[b][mc], xq2[:, q, b, :, :],
                        start=(b == 0), stop=(b == 1), perf_mode=DR,
                    )
        for q in range(2):
            if b1_zero:
                # merged relu eviction (b1 == 0), alternating engine per tile
                if q == 0:
                    nc.scalar.activation(
                        h1Ts[q][:, :, 0:R], pms[q][:, :, 0:R], RELU)
                else:
                    nc.vector.tensor_scalar(
                        h1Ts[q][:, :, 0:R], pms[q][:, :, 0:R], 0.0, None, op0=MAX)
            else:
                for mc in range(2):
                    if q == 0:
                        nc.scalar.activation(
                            h1Ts[q][:, mc, 0:R], pms[q][:, mc, 0:R], RELU,
                            bias=b1c[mc])
                    else:
                        nc.vector.tensor_scalar(
                            h1Ts[q][:, mc, 0:R], pms[q][:, mc, 0:R], b1c[mc],
                            0.0, op0=ADD, op1=MAX,
                        )
        h1Ts_of[pr] = h1Ts

    def stage_mm2(pr, R=R_TILE):
        # mm2 (K=256): even tile uses fp8 DoubleRow (dst partitions 0-63
        # is DoubleRow-legal), odd tile uses two bf16 matmuls into 64-127;
        # then e = exp(svec*p2 + b2) on ACT.
        h1Ts = h1Ts_of.pop(pr)
        p2 = h2_psum.tile([P, R_TILE], f32, name="h2p", tag="h2p")
        nc.tensor.matmul(
            p2[0:64, 0:R], w2f8, h1Ts[0][:, :, 0:R],
            start=True, stop=True, perf_mode=DR,
        )
        for mc in range(2):
            nc.tensor.matmul(
                p2[64:128, 0:R], w2c[mc], h1Ts[1][:, mc, 0:R],
                start=(mc == 0), stop=(mc == 1),
            )
        eT = e_pool.tile([P, R_TILE], bf16, name="eT", tag="eT")
        nc.scalar.activation(eT[:, 0:R], p2[:, 0:R], EXP, bias=b2t, scale=svec)
        p2_of[pr], eT_of[pr] = p2, eT

    def stage_tail(pr, R=R_TILE):
        #   S = ones.T @ e  per 64-partition half (concurrent PE quadrants)
        #   out = svec*p2 (+ b2) - ln(S)
        p2, eT = p2_of.pop(pr), eT_of.pop(pr)
        pS = s_psum.tile([P, R_TILE], f32, name="pS", tag="pS")
        nc.tensor.matmul(pS[0:64, 0:R], ones_b[0:64, 0:64], eT[0:64, 0:R],
                         start=True, stop=True)
        nc.tensor.matmul(pS[64:128, 0:R], ones_b[64:128, 0:64],
                         eT[64:128, 0:R], start=True, stop=True)
        lsb = ls_pool.tile([P, R_TILE], f32, name="lsb", tag="lsb")
        nc.scalar.activation(lsb[:, 0:R], pS[:, 0:R], LN)
        if b2_nonzero:
            # general-b2: out = svec*p2 + b2 - ln(S) = svec*p2 - (ln(S) - b2)
            lsb2 = ls_pool.tile([P, R_TILE], f32, name="lsb2", tag="lsb2")
            nc.vector.tensor_scalar(lsb2[:, 0:R], lsb[:, 0:R], b2t, None, op0=SUB)
            lsb = lsb2
        oT = o_pool.tile([P, R_TILE], bf16, name="oT", tag="oT")
        nc.vector.scalar_tensor_tensor(
            oT[:, 0:R], p2[:, 0:R], svec, lsb[:, 0:R], op0=MULT, op1=SUB,
        )
        nc.sync.dma_start(outT_d[:, pr * R_TILE:pr * R_TILE + R], oT[:, 0:R])

    # HAM warm-up: dependency-free N=512 matmuls keep the PE busy through
    # the ~3.4us HAM activity window while the input DMAs land, so real
    # matmuls run at 2.4 GHz instead of the cold 1.2 GHz K=4/8 clock.
    wu = s_psum.tile([P, R_TILE], f32, name="wu", tag="wu")

    def warm(n):
        for _ in range(n):
            nc.tensor.matmul(wu[0:64, :], ones_b[0:64, 0:64], warm_rhs[:],
                             start=True, stop=True)

    # dependency-free padding matmuls keep the PE busy until the first
    # input DMAs land (in-order queue: padding must PRECEDE the consumer)
    # so the HAM activity window never sees an idle gap and the clock is
    # warm when real work begins.
    PAD = {0: 14, 1: 6, 2: 4, 3: 2}
    STAGE_DEPTH = int(os.environ.get("BERN_STAGE_DEPTH", "2"))
    if STAGE_DEPTH == 3:
        for pr in range(PAIRS_PER_CORE + 2):
            warm(PAD.get(pr, 0))
            if pr < PAIRS_PER_CORE:
                stage_mm1(pr)
            if 1 <= pr < PAIRS_PER_CORE + 1:
                stage_mm2(pr - 1)
            if pr >= 2:
                stage_tail(pr - 2)
    else:
        # tail(pr-1) sits between mm1(pr) and mm2(pr) in the PE stream: the
        # two ones-matmuls fill most of the window where mm2(pr) would
        # otherwise stall waiting for the relu eviction of pair pr.
        LAST = PAIRS_PER_CORE - 1
        for pr in range(PAIRS_PER_CORE + 1):
            warm(PAD.get(pr, 0))
            Rp = R_LAST if pr == LAST else R_TILE
            if pr < PAIRS_PER_CORE:
                stage_mm1(pr, Rp)
                if pr >= 1:
                    stage_tail(pr - 1, R_LAST if pr - 1 == LAST else R_TILE)
                stage_mm2(pr, Rp)
            else:
                stage_tail(pr - 1, R_LAST if pr - 1 == LAST else R_TILE)


def _build_program(b1_zero: bool = True, b2_nonzero: bool = False) -> bass.Bass:
    key = f"v12_{b1_zero}_{b2_nonzero}_{os.environ.get('BERN_STAGE_DEPTH', '2')}"
    if key in _PROGRAM_CACHE:
        return _PROGRAM_CACHE[key]
    f32 = mybir.dt.float32
    f8 = mybir.dt.float8e4
    bf16 = mybir.dt.bfloat16
    nc = _Bacc("TRN2", target_bir_lowering=False, debug=False)
    x_in = nc.dram_tensor(
        "x8", [PAIRS_PER_CORE - 1, P, 2, 2, 2, R_TILE], f8, kind="ExternalInput"
    ).ap()
    xt_in = nc.dram_tensor(
        "x8t", [P, 2, 2, 2, R_LAST], f8, kind="ExternalInput"
    ).ap()
    w1_in = nc.dram_tensor(
        "wblob", [P, BLOB_BYTES], mybir.dt.uint8, kind="ExternalInput"
    ).ap()
    b1_in = w2_in = b2_in = None
    outT_d = nc.dram_tensor(
        "outT", [P, (PAIRS_PER_CORE - 1) * R_TILE + R_LAST], bf16,
        kind="ExternalOutput"
    ).ap()
    with ExitStack() as ctx:
        tc = ctx.enter_context(tile.TileContext(nc))
        _emit(nc, tc, ctx, (x_in, xt_in, w1_in, b1_in, w2_in, b2_in, outT_d), b1_zero, b2_nonzero)
    nc.compile()
    _PROGRAM_CACHE[key] = nc
    return nc


def _pow2_floor(v: float) -> float:
    return float(2.0 ** np.floor(np.log2(v)))


def _scales(x, W1, b1):
    """Power-of-two prescales keeping all fp8 casts inside +-200.

    The product cap SX*SW is set from the exact |x@W1 + b1| max (one host
    BLAS matmul) so h1p = SX*SW*(x@W1+b1) provably stays in fp8 range.
    Within the cap, SW gets as much headroom as possible: W1 is the
    small-magnitude operand (sigma ~0.04) and suffers most from fp8
    subnormal truncation below 2^-6.
    """
    xmax = float(np.abs(x).max())
    SX0 = _pow2_floor(200.0 / xmax)
    h1max = float(np.abs(x @ W1 + b1).max())
    prod_cap = _pow2_floor(176.0 / h1max)
    w1max = float(np.abs(W1).max())
    SW_cap = _pow2_floor(200.0 / w1max)
    SX = min(SX0, max(2.0, prod_cap / SW_cap))
    SW = min(SW_cap, prod_cap / SX)
    return SX, SW


def make_in_maps(inputs: dict) -> list[dict]:
    """Host-side shard/quantize/pack into per-core device input maps."""
    x = np.asarray(inputs["x"], dtype=np.float32)
    W1 = np.asarray(inputs["W1"], dtype=np.float32)
    b1 = np.asarray(inputs["b1"], dtype=np.float32)
    W2 = np.asarray(inputs["W2"], dtype=np.float32)
    b2 = np.asarray(inputs["b2"], dtype=np.float32)

    SX, SW = _scales(x, W1, b1)

    n_pad = R_CORE * N_CORES
    xp = np.zeros((n_pad, F_IN), np.float32)
    xp[:N_NODES] = x
    xq = (xp * SX).astype(E4)
    # [core, pair, q, rr, b, j, p] -> [core][pair, p, q, b, j, rr]
    xc = xq.reshape(N_CORES, R_CORE, 2, 2, P)
    full = 12 * 2 * R_TILE
    xv = xc[:, :full].reshape(N_CORES, 12, 2, R_TILE, 2, 2, P)
    xtv = xc[:, full:].reshape(N_CORES, 2, R_LAST, 2, 2, P)

    # pack every weight/bias into one per-partition byte blob (see _emit)
    blob = np.zeros((P, BLOB_BYTES), np.uint8)
    w1q = (W1 * SW).astype(E4)
    w1r = w1q.reshape(2, 2, P, 2, P)          # [b, j, p, mc, m]
    for b in range(2):
        for mc in range(2):
            chunk = w1r[b, :, :, mc, :].transpose(1, 0, 2).reshape(P, 256)
            blob[:, (2 * b + mc) * 256:(2 * b + mc + 1) * 256] = \
                chunk.view(np.uint8)
    w2t = (W2 / (SX * SW)).astype(BF).reshape(2, P, F_OUT)
    for mc in range(2):
        blob[:, 1024 + mc * 128:1024 + (mc + 1) * 128] = \
            np.ascontiguousarray(w2t[mc]).view(np.uint8)
    b1s = ((SX * SW) * b1).astype(np.float32)
    blob[:, 1280:1284] = b1s[:P].view(np.uint8).reshape(P, 4)
    blob[:, 1284:1288] = b1s[P:].view(np.uint8).reshape(P, 4)
    blob[:, 1288:1292] = np.tile(b2, 2).astype(np.float32).view(np.uint8).reshape(P, 4)
    w2max = float(np.abs(W2).max())
    SW2 = _pow2_floor(200.0 / w2max)
    w2q8 = (W2 * SW2).astype(E4).reshape(2, P, F_OUT).transpose(1, 0, 2)
    blob[:, 1292:1420] = np.ascontiguousarray(w2q8).reshape(P, 128).view(np.uint8)
    svec = np.empty(P, np.float32)
    svec[:64] = 1.0 / (SX * SW * SW2)
    svec[64:] = 1.0
    blob[:, 1420:1424] = svec.view(np.uint8).reshape(P, 4)

    in_maps = [
        {
            "x8": np.ascontiguousarray(xv[i].transpose(0, 5, 1, 3, 4, 2)),
            "x8t": np.ascontiguousarray(xtv[i].transpose(4, 0, 2, 3, 1)),
            "wblob": blob,
        }
        for i in range(N_CORES)
    ]
    return in_maps


def unpack_out(results: list[dict]) -> np.ndarray:
    cores = []
    for i in range(N_CORES):
        oT = np.asarray(results[i]["outT"]).astype(np.float32)
        vf = oT[:, :12 * R_TILE].reshape(2, F_OUT, 12, R_TILE)
        head = vf.transpose(2, 0, 3, 1).reshape(12 * 2 * R_TILE, F_OUT)
        vt = oT[:, 12 * R_TILE:].reshape(2, F_OUT, R_LAST)
        tail = vt.transpose(0, 2, 1).reshape(2 * R_LAST, F_OUT)
        cores.append(np.concatenate([head, tail], axis=0))
    out = np.concatenate(cores, axis=0)
    return np.ascontiguousarray(out[:N_NODES])


def _bern_alpha(theta: np.ndarray) -> np.ndarray:
    """Coefficients alpha_j of sum_k theta_k C(K,k)/2^K (1-t)^k (1+t)^{K-k}."""
    alpha = np.zeros(KBERN + 1, dtype=np.float64)
    for k in range(KBERN + 1):
        poly = np.array([1.0])
        for _ in range(k):
            poly = np.convolve(poly, [1.0, -1.0])
        for _ in range(KBERN - k):
            poly = np.convolve(poly, [1.0, 1.0])
        alpha += (comb(KBERN, k) / 2.0 ** KBERN) * float(theta[k]) * poly
    return alpha


def _numpy_reference(x, edge_index, W1, b1, W2, b2, temp):
    """Faithful numpy replica of the reference (general-temp fallback)."""
    n = x.shape[0]
    h = np.maximum(x @ W1 + b1, 0.0).astype(np.float32)
    h = (h @ W2 + b2).astype(np.float32)
    theta = np.maximum(temp.astype(np.float32), 0.0)
    row, col = edge_index[0], edge_index[1]
    deg = np.zeros(n, np.float32)
    np.add.at(deg, row, np.float32(1.0))
    dinv = np.where(deg > 0, 1.0 / np.sqrt(deg), 0.0).astype(np.float32)
    w = (dinv[row] * dinv[col])[:, None].astype(np.float32)

    def adj(v):
        out = np.zeros_like(v)
        np.add.at(out, row, v[col] * w)
        return out

    tmp = [h]
    v = h
    for _ in range(KBERN):
        v = v + adj(v)
        tmp.append(v)
    scale = np.float32(1.0 / 2.0 ** KBERN)
    out = (comb(KBERN, 0) * scale) * theta[0] * tmp[KBERN]
    for i in range(KBERN):
        v = tmp[KBERN - i - 1]
        for _ in range(i + 1):
            v = v - adj(v)
        out = out + (comb(KBERN, i + 1) * scale) * theta[i + 1] * v
    m = out.max(axis=1, keepdims=True)
    ex = np.exp(out - m)
    return ((out - m) - np.log(ex.sum(axis=1, keepdims=True))).astype(np.float32)


def kernel(**inputs) -> np.ndarray:
    x = np.asarray(inputs["x"], dtype=np.float32)
    W1 = np.asarray(inputs["W1"], dtype=np.float32)
    W2 = np.asarray(inputs["W2"], dtype=np.float32)
    temp = np.asarray(inputs["temp"], dtype=np.float32)
    edge_index = np.asarray(inputs["edge_index"])

    theta = np.maximum(temp.astype(np.float64), 0.0)
    alpha = _bern_alpha(theta)
    collapses = abs(alpha[0] - 1.0) < 1e-9 and np.all(np.abs(alpha[1:]) < 1e-9)
    if not (collapses and x.shape == (N_NODES, F_IN) and W1.shape == (F_IN, F_MID)
            and W2.shape == (F_MID, F_OUT)):
        return _numpy_reference(
            x, edge_index.astype(np.int64), W1,
            np.asarray(inputs["b1"], np.float32), W2,
            np.asarray(inputs["b2"], np.float32), temp,
        )

    in_maps = make_in_maps(inputs)
    nc = _build_program(not np.any(np.asarray(inputs["b1"])),
                        bool(np.any(np.asarray(inputs["b2"]))))
    res = run_bass_kernel_spmd(nc, in_maps, list(range(N_CORES))).results
    return unpack_out(res)
